# revision 5
# baseline (speedup 1.0000x reference)
"""Trainium2 Bass kernel for nn_CliffordKANLayer (B=2048, I=128, O=128, G=8, D=2).

Math (see reference):
    rbf[b,i,u,v] = exp(-((xr-g_u)^2 + (xi-g_v)^2))            (separable!)
                 = pr[b,i,u] * pi[b,i,v]
    out[b,o,z]   = sum_{i,u,v} rbf * W[i,o,u,v,z]
                 + sum_{i,x,y} sw[i,o,x] silu(x)[b,i,y] C[x,y,z]
                 + sum_i bias[i,o,z]
    then BatchNorm over (B,O) per z.

Mapping to 8 NeuronCores (data-parallel over batch, 256 rows per core):
    - pr/pi computed on ScalarE (Square + Exp activations), partition dim = i.
    - R chunks R_uv[i,b] = pr_u[i,b]*pi_v[i,b] built on VectorE with a
      stride-0 broadcast access pattern (one op per u covers all v).
    - 64 accumulating PE matmuls per 128-row batch tile:
      psum[b,(o,z)] += R_uv^T @ W_uv, K=128(i), N=256((o,z)); plus 2 SiLU
      matmuls (cayley folded into the weight host-side) and a ones-matmul
      that adds colsum_i(bias).
    - BatchNorm stats: per-z strided free reductions + ones-matmul partition
      reduction -> (128,4) partial [s0,s1,ss0,ss1] -> 2KB AllReduce across
      the 8 cores -> affine applied with per-partition scale/bias on ScalarE.
"""

import copy
import sys

if "/opt/trn_rl_repo" not in sys.path:
    sys.path.insert(0, "/opt/trn_rl_repo")

import numpy as np

import concourse.bass as bass
import concourse.mybir as mybir
import concourse.tile as tile
from concourse.bass_utils import run_bass_kernel_spmd

B, I_DIM, O_DIM, G, D = 2048, 128, 128, 8, 2
NCORES = 8
BC = B // NCORES          # 256 batch rows per core
N_OUT = O_DIM * D         # 256 output columns (o,z)
KCH = G * G               # 64 contraction chunks of 128
EPS = 1e-5
INV_COUNT = 1.0 / (B * O_DIM)

# "f32r": fp32 data, matmuls in float32r (full-rate at N>=256, ~fp32 accuracy)
# "f32" : plain fp32 matmuls (4x slower PE, bit-safest)
# "bf16": R/W/silu operands in bf16 (half DMA, 2x DVE), psum/BN in fp32
PATH = "f32r"

F32 = mybir.dt.float32
AF = mybir.ActivationFunctionType
ALU = mybir.AluOpType

_cache = {}


class _TailSplitTileContext(tile.TileContext):
    """TileContext whose tail drain carries at most one semaphore wait per
    instruction -- this walrus build rejects >1 sync wait on CTRL ops."""

    def _drain_and_barrier(self, tick_clock, wait_clock):
        nc = self.nc
        drain_inst = nc.sync.drain().ins
        wait_clock.add_sem_waits(
            drain_inst, tile.ScopedClock({None: tick_clock.global_clock})
        )
        si = drain_inst.sync_info
        waits = list(si.on_wait) if si is not None and si.on_wait else []
        if len(waits) > 1:
            si1 = copy.deepcopy(si)
            si1.on_wait = waits[:1]
            drain_inst.sync_info = si1
            for w in waits[1:]:
                d = nc.sync.drain().ins
                si_extra = copy.deepcopy(si)
                si_extra.on_wait = [w]
                d.sync_info = si_extra
        nc.all_engine_barrier()
        popped = nc._tile_sem_poison_stack.pop()
        assert popped is self._sem_poison
        nc.clear_and_free_semaphores(list(self.sems.allocated().values()))
        nc.all_engine_barrier()


def _split_excess_waits(nc, max_waits=1):
    """Hoist surplus semaphore waits onto injected same-engine no-ops
    (the ISA encodes a single wait slot per instruction here)."""
    ctr = 0
    for f in nc.m.functions:
        for blk in f.blocks:
            insts = list(blk.instructions)
            out = []
            changed = False
            for ins in insts:
                si = ins.sync_info
                waits = list(si.on_wait) if (si is not None and si.on_wait) else []
                if len(waits) > max_waits:
                    changed = True
                    extra, keep = waits[:-max_waits], waits[-max_waits:]
                    for j in range(0, len(extra), max_waits):
                        nop = mybir.InstNoOp(name=f"wsplit_nop_{ctr}", ins=[], outs=[])
                        ctr += 1
                        nop.engine = ins.engine
                        si_n = copy.deepcopy(si)
                        si_n.on_wait = extra[j : j + max_waits]
                        if si_n.on_update:
                            si_n.on_update = []
                        nop.sync_info = si_n
                        nc.register_instruction(nop)
                        out.append(nop)
                    si_k = copy.deepcopy(si)
                    si_k.on_wait = keep
                    ins.sync_info = si_k
                out.append(ins)
            if changed:
                blk.instructions = out


def _build(path=PATH):
    """Build the SPMD Bass program (identical on all 8 cores)."""
    if path == "bf16":
        ct = mybir.dt.bfloat16
    elif path == "f32r":
        # float32r: same bits as fp32 in DRAM/SBUF, but the PE runs the
        # matmul at full rate (vs 4 cyc/row for plain fp32) when N>=256.
        # The BIR verifier requires every producer feeding an fp32r matmul
        # to carry the f32r dtype, so the dtype is threaded end-to-end.
        ct = mybir.dt.float32r
    else:
        ct = F32

    nc = bass.Bass("TRN2", target_bir_lowering=False, debug=False,
                   num_devices=NCORES)

    # --- kernel I/O (per core) ---
    xr_d = nc.dram_tensor("xr", [I_DIM, BC], F32, kind="ExternalInput")
    xi_d = nc.dram_tensor("xi", [I_DIM, BC], F32, kind="ExternalInput")
    w2_d = nc.dram_tensor("w2", [KCH, I_DIM, N_OUT], ct, kind="ExternalInput")
    ms_d = nc.dram_tensor("msil", [2, I_DIM, N_OUT], ct, kind="ExternalInput")
    bi_d = nc.dram_tensor("biasr", [I_DIM, N_OUT], ct, kind="ExternalInput")
    gr_d = nc.dram_tensor("gnr", [I_DIM, G], F32, kind="ExternalInput")
    gi_d = nc.dram_tensor("gni", [I_DIM, G], F32, kind="ExternalInput")
    gam_d = nc.dram_tensor("gam", [I_DIM, D], F32, kind="ExternalInput")
    bet_d = nc.dram_tensor("bet", [I_DIM, D], F32, kind="ExternalInput")
    on_d = nc.dram_tensor("onesw", [I_DIM, I_DIM], ct, kind="ExternalInput")
    y_d = nc.dram_tensor("y", [BC, N_OUT], F32, kind="ExternalOutput")

    # --- internal DRAM for the stats AllReduce ---
    st_loc = nc.dram_tensor("stats_loc", [I_DIM, 4], F32)
    st_sh = nc.dram_tensor("stats_sh", [I_DIM, 4], F32, addr_space="Shared")

    with _TailSplitTileContext(nc) as tc:
        with (
            tc.tile_pool(name="const", bufs=1) as cpool,
            tc.tile_pool(name="prpi", bufs=1) as ppool,
            tc.tile_pool(name="sq", bufs=2) as sqpool,
            tc.tile_pool(name="rch", bufs=3) as rpool,
            tc.tile_pool(name="wch", bufs=16) as wpool,
            tc.tile_pool(name="outp", bufs=1) as opool,
            tc.tile_pool(name="bn", bufs=1) as bnpool,
            tc.tile_pool(name="ps", bufs=1, space=bass.MemorySpace.PSUM) as pspool,
        ):
            # ---- input loads ----
            xr = cpool.tile([I_DIM, BC], F32, tag="xr")
            nc.sync.dma_start(xr[:], xr_d.ap())
            xi = cpool.tile([I_DIM, BC], F32, tag="xi")
            nc.sync.dma_start(xi[:], xi_d.ap())
            gnr = cpool.tile([I_DIM, G], F32, tag="gnr")
            nc.sync.dma_start(gnr[:], gr_d.ap())
            gni = cpool.tile([I_DIM, G], F32, tag="gni")
            nc.sync.dma_start(gni[:], gi_d.ap())
            gam = cpool.tile([I_DIM, D], F32, tag="gam")
            nc.sync.dma_start(gam[:], gam_d.ap())
            bet = cpool.tile([I_DIM, D], F32, tag="bet")
            nc.sync.dma_start(bet[:], bet_d.ap())
            m0 = cpool.tile([I_DIM, N_OUT], ct, tag="m0")
            nc.sync.dma_start(m0[:], ms_d.ap()[0])
            m1 = cpool.tile([I_DIM, N_OUT], ct, tag="m1")
            nc.sync.dma_start(m1[:], ms_d.ap()[1])
            biasr = cpool.tile([I_DIM, N_OUT], ct, tag="biasr")
            nc.sync.dma_start(biasr[:], bi_d.ap())

            ones = cpool.tile([I_DIM, I_DIM], ct, tag="ones")
            nc.sync.dma_start(ones[:], on_d.ap())
            ones_f = cpool.tile([I_DIM, I_DIM], F32, tag="ones_f")
            nc.gpsimd.memset(ones_f[:], 1.0)

            # ---- SiLU branch operands (partition=i, free=b) ----
            # silu(x) = x * sigmoid(x); composed so sim and HW share the path
            sg = sqpool.tile([I_DIM, BC], F32, tag="sg")
            s0 = cpool.tile([I_DIM, BC], ct, tag="s0")
            nc.scalar.activation(sg[:], xr[:], AF.Sigmoid)
            nc.vector.tensor_mul(s0[:], sg[:], xr[:])
            sg2 = sqpool.tile([I_DIM, BC], F32, tag="sg")
            s1 = cpool.tile([I_DIM, BC], ct, tag="s1")
            nc.scalar.activation(sg2[:], xi[:], AF.Sigmoid)
            nc.vector.tensor_mul(s1[:], sg2[:], xi[:])

            # ---- pr/pi:  exp(-(x - g)^2) for the 8 grid points each ----
            pr = ppool.tile([I_DIM, G, BC], ct, tag="pr")
            pi = ppool.tile([I_DIM, G, BC], ct, tag="pi")
            for u in range(G):
                sq = sqpool.tile([I_DIM, BC], F32, tag="sq")
                nc.scalar.activation(sq[:], xr[:], AF.Square,
                                     bias=gnr[:, u : u + 1])
                nc.scalar.activation(pr[:, u, :], sq[:], AF.Exp, scale=-1.0)
            for v in range(G):
                sq = sqpool.tile([I_DIM, BC], F32, tag="sq")
                nc.scalar.activation(sq[:], xi[:], AF.Square,
                                     bias=gni[:, v : v + 1])
                nc.scalar.activation(pi[:, v, :], sq[:], AF.Exp, scale=-1.0)

            # ---- W chunk stream ----
            wts = []
            for k in range(KCH):
                wt = wpool.tile([I_DIM, N_OUT], ct, tag="w")
                nc.sync.dma_start(wt[:], w2_d.ap()[k])
                wts.append(wt)

            # ---- main contraction: psum[b, (o,z)] over 2 batch halves ----
            ps0 = pspool.tile([128, N_OUT], F32, tag="ps0")
            ps1 = pspool.tile([128, N_OUT], F32, tag="ps1")
            for u in range(G):
                r = rpool.tile([I_DIM, G, BC], ct, tag="r")
                nc.vector.tensor_mul(
                    r[:],
                    pr[:, u : u + 1, :].broadcast_to((I_DIM, G, BC)),
                    pi[:],
                )
                for v in range(G):
                    k = u * G + v
                    nc.tensor.matmul(ps0[:], r[:, v, 0:128],
                                     wts[k][:],
                                     start=(k == 0), stop=False)
                    nc.tensor.matmul(ps1[:], r[:, v, 128:256],
                                     wts[k][:],
                                     start=(k == 0), stop=False)

            # ---- SiLU/cayley + bias-colsum matmuls ----
            nc.tensor.matmul(ps0[:], s0[:, 0:128], m0[:],
                             start=False, stop=False)
            nc.tensor.matmul(ps1[:], s0[:, 128:256], m0[:],
                             start=False, stop=False)
            nc.tensor.matmul(ps0[:], s1[:, 0:128], m1[:],
                             start=False, stop=False)
            nc.tensor.matmul(ps1[:], s1[:, 128:256], m1[:],
                             start=False, stop=False)
            nc.tensor.matmul(ps0[:], ones[:], biasr[:],
                             start=False, stop=True)
            nc.tensor.matmul(ps1[:], ones[:], biasr[:],
                             start=False, stop=True)

            # ---- BatchNorm partials: [sum_z0, sum_z1, sumsq_z0, sumsq_z1] ----
            st0 = bnpool.tile([128, 4], F32, tag="st0")
            st1 = bnpool.tile([128, 4], F32, tag="st1")
            for zi, (pst, stt) in enumerate(((ps0, st0), (ps1, st1))):
                zview = pst[:].rearrange("p (o z) -> p z o", z=D)
                for z in range(D):
                    nc.vector.tensor_reduce(stt[:, z : z + 1], zview[:, z, :],
                                            axis=mybir.AxisListType.X,
                                            op=ALU.add)
                    sqz = sqpool.tile([128, O_DIM], F32, tag="sqz")
                    nc.scalar.activation(sqz[:], zview[:, z, :], AF.Square)
                    nc.vector.tensor_reduce(stt[:, 2 + z : 3 + z], sqz[:],
                                            axis=mybir.AxisListType.X,
                                            op=ALU.add)

            # partition-sum via ones matmul (every output row = total)
            stp = pspool.tile([128, 4], F32, tag="stp")
            nc.tensor.matmul(stp[:], ones_f[:], st0[:], start=True, stop=False)
            nc.tensor.matmul(stp[:], ones_f[:], st1[:], start=False, stop=True)
            stloc = bnpool.tile([128, 4], F32, tag="stloc")
            nc.vector.tensor_copy(stloc[:], stp[:])
            nc.sync.dma_start(st_loc.ap(), stloc[:])

            # ---- 2KB AllReduce of the partial stats across the 8 cores ----
            stred = bnpool.tile([128, 4], F32, tag="stred")
            with tc.tile_critical():
                cc_sem = nc.alloc_semaphore("cc_done")
                cc_dma_sem = nc.alloc_semaphore("cc_dma")
                nc.gpsimd.collective_compute(
                    "AllReduce",
                    ALU.add,
                    replica_groups=[list(range(NCORES))],
                    ins=[st_loc.ap()],
                    outs=[st_sh.ap()],
                ).then_inc(cc_sem)
                nc.gpsimd.wait_ge(cc_sem, 1)
                nc.gpsimd.dma_start(stred[:], st_sh.ap()).then_inc(cc_dma_sem, 16)
                nc.gpsimd.wait_ge(cc_dma_sem, 16)

            # ---- scale/shift per z (on all 128 partitions) ----
            mean = bnpool.tile([128, D], F32, tag="mean")
            nc.vector.tensor_scalar_mul(mean[:], stred[:, 0:2], INV_COUNT)
            msq = bnpool.tile([128, D], F32, tag="msq")
            nc.vector.tensor_scalar_mul(msq[:], stred[:, 2:4], INV_COUNT)
            var = bnpool.tile([128, D], F32, tag="var")
            nc.vector.tensor_mul(var[:], mean[:], mean[:])
            nc.vector.tensor_sub(var[:], msq[:], var[:])
            nc.vector.tensor_scalar_add(var[:], var[:], EPS)
            inv = bnpool.tile([128, D], F32, tag="inv")
            nc.vector.reciprocal(inv[:], var[:])
            nc.scalar.activation(inv[:], inv[:], AF.Sqrt)   # 1/std
            scl = bnpool.tile([128, D], F32, tag="scl")
            nc.vector.tensor_mul(scl[:], gam[:], inv[:])
            shf = bnpool.tile([128, D], F32, tag="shf")
            nc.vector.tensor_mul(shf[:], mean[:], scl[:])
            nc.vector.tensor_sub(shf[:], bet[:], shf[:])

            # ---- apply + store ----
            for bh, pst in enumerate((ps0, ps1)):
                ot = opool.tile([128, N_OUT], F32, tag=f"out{bh}")
                pv = pst[:].rearrange("p (o z) -> p z o", z=D)
                ov = ot[:].rearrange("p (o z) -> p z o", z=D)
                for z in range(D):
                    nc.scalar.activation(ov[:, z, :], pv[:, z, :], AF.Identity,
                                         bias=shf[:, z : z + 1],
                                         scale=scl[:, z : z + 1])
                nc.sync.dma_start(y_d.ap()[bh * 128 : (bh + 1) * 128, :], ot[:])

    _split_excess_waits(nc)
    return nc


def _prep_inputs(x, weights, silu_weight, silu_bias, gamma, beta, grid, cayley,
                 path=PATH):
    """Host-side sharding + operand layout (no math beyond folding the tiny
    cayley table into the silu weight)."""
    if path == "bf16":
        import ml_dtypes
        ctnp = ml_dtypes.bfloat16
    else:
        ctnp = np.float32

    x = np.asarray(x, np.float32)
    w2 = np.ascontiguousarray(
        np.transpose(np.asarray(weights, np.float32), (2, 3, 0, 1, 4))
    ).reshape(KCH, I_DIM, N_OUT).astype(ctnp)
    msil = np.einsum("iox,xyz->yioz", np.asarray(silu_weight, np.float32),
                     np.asarray(cayley, np.float32)).reshape(2, I_DIM, N_OUT)
    msil = np.ascontiguousarray(msil).astype(ctnp)
    biasr = np.asarray(silu_bias, np.float32).reshape(I_DIM, N_OUT).astype(ctnp)
    gnr = np.tile(-np.asarray(grid, np.float32)[:, 0, 0], (I_DIM, 1))
    gni = np.tile(-np.asarray(grid, np.float32)[0, :, 1], (I_DIM, 1))
    onesw = np.ones((I_DIM, I_DIM), np.float32).astype(ctnp)
    gam = np.tile(np.asarray(gamma, np.float32), (I_DIM, 1))
    bet = np.tile(np.asarray(beta, np.float32), (I_DIM, 1))

    in_maps = []
    for c in range(NCORES):
        xs = x[c * BC : (c + 1) * BC]          # (BC, I, 2)
        in_maps.append({
            "xr": np.ascontiguousarray(xs[:, :, 0].T),
            "xi": np.ascontiguousarray(xs[:, :, 1].T),
            "w2": w2,
            "msil": msil,
            "biasr": biasr,
            "onesw": onesw,
            "gnr": gnr,
            "gni": gni,
            "gam": gam,
            "bet": bet,
        })
    return in_maps


def kernel(x, weights, silu_weight, silu_bias, gamma, beta, grid, cayley):
    if "nc" not in _cache:
        _cache["nc"] = _build(PATH)
    nc = _cache["nc"]
    in_maps = _prep_inputs(x, weights, silu_weight, silu_bias, gamma, beta,
                           grid, cayley, PATH)
    res = run_bass_kernel_spmd(nc, in_maps, core_ids=list(range(NCORES)))
    y = np.concatenate([res.results[c]["y"] for c in range(NCORES)], axis=0)
    return y.reshape(B, O_DIM, D)


# revision 18
# speedup vs baseline: 2.3211x; 2.3211x over previous
"""Trainium2 Bass kernel for nn_CliffordKANLayer (B=2048, I=128, O=128, G=8, D=2).

Math (see reference):
    rbf[b,i,u,v] = exp(-((xr-g_u)^2 + (xi-g_v)^2))            (separable!)
                 = pr[b,i,u] * pi[b,i,v]
    out[b,o,z]   = sum_{i,u,v} rbf * W[i,o,u,v,z]
                 + sum_{i,x,y} sw[i,o,x] silu(x)[b,i,y] C[x,y,z]
                 + sum_i bias[i,o,z]
    then BatchNorm over (B,O) per z.

Mapping to 8 NeuronCores (data-parallel over batch, 256 rows per core):
    - pr/pi computed on ScalarE (Square + Exp activations), partition dim = i.
    - R chunks R_uv[i,b] = pr_u[i,b]*pi_v[i,b] built on VectorE with a
      stride-0 broadcast access pattern (one op per u covers all v).
    - 64 accumulating PE matmuls per 128-row batch tile:
      psum[b,(o,z)] += R_uv^T @ W_uv, K=128(i), N=256((o,z)); plus 2 SiLU
      matmuls (cayley folded into the weight host-side) and a ones-matmul
      that adds colsum_i(bias).
    - BatchNorm stats: per-z strided free reductions + ones-matmul partition
      reduction -> per-core partial [s0,s1,ss0,ss1].
    - Cross-core stats combine (default TWO_PHASE=True): phase 1 returns the
      raw pre-norm outputs + 4 partial sums per core; the host adds the 8x4
      floats and launches a tiny affine phase-2 kernel. This sidesteps
      collective_compute, whose per-execution setup floor (~60-90us) made a
      2KB on-device AllReduce cost more than the whole RBF contraction.
      TWO_PHASE=False keeps the single-launch on-device AllReduce variant.
"""

import copy
import sys

if "/opt/trn_rl_repo" not in sys.path:
    sys.path.insert(0, "/opt/trn_rl_repo")

import numpy as np

import concourse.bass as bass
import concourse.mybir as mybir
import concourse.tile as tile
from concourse.bass_utils import run_bass_kernel_spmd

B, I_DIM, O_DIM, G, D = 2048, 128, 128, 8, 2
NCORES = 8
BC = B // NCORES          # 256 batch rows per core
N_OUT = O_DIM * D         # 256 output columns (o,z)
KCH = G * G               # 64 contraction chunks of 128
EPS = 1e-5
INV_COUNT = 1.0 / (B * O_DIM)

# "f32r": fp32 data, matmuls in float32r (full-rate at N>=256, ~fp32 accuracy)
# "f32" : plain fp32 matmuls (4x slower PE, bit-safest)
# "bf16": R/W/silu operands in bf16 (half DMA, 2x DVE), psum/BN in fp32
PATH = "f32r"

# True: two SPMD launches with the 32-float BatchNorm-stats reduction done
# on the host between them -- avoids collective_compute entirely, whose
# per-execution setup/barrier floor (~60-90us before the transfer may even
# start) dominates the single-launch version. False: single launch with an
# on-device AllReduce.
TWO_PHASE = True

F32 = mybir.dt.float32
AF = mybir.ActivationFunctionType
ALU = mybir.AluOpType

_cache = {}


class _TailSplitTileContext(tile.TileContext):
    """TileContext whose tail drain carries at most one semaphore wait per
    instruction -- this walrus build rejects >1 sync wait on CTRL ops."""

    def _drain_and_barrier(self, tick_clock, wait_clock):
        nc = self.nc
        drain_inst = nc.sync.drain().ins
        wait_clock.add_sem_waits(
            drain_inst, tile.ScopedClock({None: tick_clock.global_clock})
        )
        si = drain_inst.sync_info
        waits = list(si.on_wait) if si is not None and si.on_wait else []
        if len(waits) > 1:
            si1 = copy.deepcopy(si)
            si1.on_wait = waits[:1]
            drain_inst.sync_info = si1
            for w in waits[1:]:
                d = nc.sync.drain().ins
                si_extra = copy.deepcopy(si)
                si_extra.on_wait = [w]
                d.sync_info = si_extra
        nc.all_engine_barrier()
        popped = nc._tile_sem_poison_stack.pop()
        assert popped is self._sem_poison
        nc.clear_and_free_semaphores(list(self.sems.allocated().values()))
        nc.all_engine_barrier()


def _split_excess_waits(nc, max_waits=1):
    """Hoist surplus semaphore waits onto injected same-engine no-ops
    (the ISA encodes a single wait slot per instruction here)."""
    ctr = 0
    for f in nc.m.functions:
        for blk in f.blocks:
            insts = list(blk.instructions)
            out = []
            changed = False
            for ins in insts:
                si = ins.sync_info
                waits = list(si.on_wait) if (si is not None and si.on_wait) else []
                if len(waits) > max_waits:
                    changed = True
                    extra, keep = waits[:-max_waits], waits[-max_waits:]
                    for j in range(0, len(extra), max_waits):
                        nop = mybir.InstNoOp(name=f"wsplit_nop_{ctr}", ins=[], outs=[])
                        ctr += 1
                        nop.engine = ins.engine
                        si_n = copy.deepcopy(si)
                        si_n.on_wait = extra[j : j + max_waits]
                        if si_n.on_update:
                            si_n.on_update = []
                        nop.sync_info = si_n
                        nc.register_instruction(nop)
                        out.append(nop)
                    si_k = copy.deepcopy(si)
                    si_k.on_wait = keep
                    ins.sync_info = si_k
                out.append(ins)
            if changed:
                blk.instructions = out


def _build(path=PATH, two_phase=False):
    """Build the SPMD Bass program (identical on all 8 cores)."""
    nc = _build_inner(path, two_phase)
    _split_excess_waits(nc)
    return nc


def _build_inner(path, two_phase):
    if path == "bf16":
        ct = mybir.dt.bfloat16
    elif path == "f32r":
        # float32r: same bits as fp32 in DRAM/SBUF, but the PE runs the
        # matmul at full rate (vs 4 cyc/row for plain fp32) when N>=256.
        # The BIR verifier requires every producer feeding an fp32r matmul
        # to carry the f32r dtype, so the dtype is threaded end-to-end.
        ct = mybir.dt.float32r
    else:
        ct = F32

    nc = bass.Bass("TRN2", target_bir_lowering=False, debug=False,
                   num_devices=NCORES)

    # --- kernel I/O (per core) ---
    # xri: [xr; xi] stacked; cpack: [-gr | -gi | gamma | beta] columns;
    # msb: [m0; m1; bias_r] stacked -- packing cuts the serial const-DMA
    # chain at kernel start from 10 transfers to 4.
    xri_d = nc.dram_tensor("xri", [2, I_DIM, BC], F32, kind="ExternalInput")
    w2_d = nc.dram_tensor("w2", [KCH, I_DIM, N_OUT], ct, kind="ExternalInput")
    msb_d = nc.dram_tensor("msb", [3, I_DIM, N_OUT], ct, kind="ExternalInput")
    cp_d = nc.dram_tensor("cpack", [I_DIM, 2 * G + 2 * D, 1], F32,
                          kind="ExternalInput")
    on_d = nc.dram_tensor("onesw", [I_DIM, I_DIM], ct, kind="ExternalInput")
    y_d = nc.dram_tensor("y", [BC, N_OUT], F32, kind="ExternalOutput")

    if two_phase:
        st_d = nc.dram_tensor("stats", [1, 4], F32, kind="ExternalOutput")
    else:
        # --- internal DRAM for the stats AllReduce ---
        st_loc = nc.dram_tensor("stats_loc", [I_DIM, 4], F32)
        st_sh = nc.dram_tensor("stats_sh", [I_DIM, 4], F32, addr_space="Shared")

    with _TailSplitTileContext(nc) as tc:
        with (
            tc.tile_pool(name="const", bufs=1) as cpool,
            tc.tile_pool(name="prpi", bufs=1) as ppool,
            tc.tile_pool(name="sq", bufs=2) as sqpool,
            tc.tile_pool(name="rch", bufs=3) as rpool,
            tc.tile_pool(name="wch", bufs=4) as wpool,
            tc.tile_pool(name="outp", bufs=1) as opool,
            tc.tile_pool(name="bn", bufs=1) as bnpool,
            tc.tile_pool(name="ps", bufs=1, space=bass.MemorySpace.PSUM) as pspool,
        ):
            # ---- input loads (4 packed transfers on 3 queues) ----
            xri = cpool.tile([I_DIM, 2, BC], F32, tag="xri")
            nc.sync.dma_start(xri[:], xri_d.ap().rearrange("c p b -> p c b"))
            cp = cpool.tile([I_DIM, 2 * G + 2 * D, 1], F32, tag="cp")
            nc.gpsimd.dma_start(cp[:], cp_d.ap())
            msb = cpool.tile([I_DIM, 3, N_OUT], ct, tag="msb")
            nc.scalar.dma_start(msb[:], msb_d.ap().rearrange("c p n -> p c n"))
            ones = cpool.tile([I_DIM, I_DIM], ct, tag="ones")
            nc.gpsimd.dma_start(ones[:], on_d.ap())
            ones_f = cpool.tile([I_DIM, I_DIM], F32, tag="ones_f")
            nc.gpsimd.memset(ones_f[:], 1.0)
            xr = xri[:, 0, :]
            xi = xri[:, 1, :]
            m0 = msb[:, 0, :]
            m1 = msb[:, 1, :]
            biasr = msb[:, 2, :]
            gam = cp[:, 2 * G : 2 * G + D, 0]
            bet = cp[:, 2 * G + D : 2 * G + 2 * D, 0]

            # ---- SiLU branch operands (partition=i, free=b) ----
            # silu(x) = x * sigmoid(x); composed so sim and HW share the path
            sg = sqpool.tile([I_DIM, BC], F32, tag="sg")
            s0 = cpool.tile([I_DIM, BC], ct, tag="s0")
            nc.scalar.activation(sg[:], xr, AF.Sigmoid)
            nc.vector.tensor_mul(s0[:], sg[:], xr)
            sg2 = sqpool.tile([I_DIM, BC], F32, tag="sg")
            s1 = cpool.tile([I_DIM, BC], ct, tag="s1")
            nc.scalar.activation(sg2[:], xi, AF.Sigmoid)
            nc.vector.tensor_mul(s1[:], sg2[:], xi)

            # ---- pr/pi:  exp(-(x - g)^2) for the 8 grid points each.
            # pi is on every R chunk's critical path, so it is produced by
            # three wide ops (a DVE subtract using broadcast APs + one
            # Square + one Exp across all 8 grid points, amortizing per-op
            # engine overhead); pr is produced per-u so R_u unblocks
            # incrementally right behind it.
            pr = ppool.tile([I_DIM, G, BC], ct, tag="pr")
            pi = ppool.tile([I_DIM, G, BC], ct, tag="pi")
            di = ppool.tile([I_DIM, G, BC], F32, tag="di")
            nc.vector.scalar_tensor_tensor(
                di[:],
                xri[:, 1:2, :].broadcast_to((I_DIM, G, BC)),
                1.0,
                cp[:, G : 2 * G, :].broadcast_to((I_DIM, G, BC)),
                op0=ALU.mult,
                op1=ALU.add,
            )
            nc.scalar.activation(di[:], di[:], AF.Square)
            nc.scalar.activation(pi[:], di[:], AF.Exp, scale=-1.0)
            for u in range(G):
                sq = sqpool.tile([I_DIM, BC], F32, tag="sq")
                nc.scalar.activation(sq[:], xri[:, 0, :], AF.Square,
                                     bias=cp[:, u, :])
                nc.scalar.activation(pr[:, u, :], sq[:], AF.Exp, scale=-1.0)

            # ---- W chunk stream: 8 chunks per DMA (4KB per partition
            # line), round-robin over the 3 queue-owning engines so the
            # transfers overlap instead of serializing on one HWDGE queue ----
            dma_engs = [nc.sync, nc.scalar, nc.gpsimd]
            WQN = 8                       # chunks per W transfer
            wqs = []
            for q in range(KCH // WQN):
                wq = wpool.tile([I_DIM, WQN, N_OUT], ct, tag="w")
                src_ap = w2_d.ap()[WQN * q : WQN * (q + 1)].rearrange(
                    "c p n -> p c n")
                dma_engs[q % len(dma_engs)].dma_start(wq[:], src_ap)
                wqs.append(wq)

            # ---- main contraction: psum[b, (o,z)] over 2 batch halves ----
            # The SiLU/cayley + bias matmuls open each accumulation group:
            # their operands are ready within ~3us, so the PE starts (and the
            # HAM clock-gate warms) long before the first RBF chunk lands.
            ps0 = pspool.tile([128, N_OUT], F32, tag="ps0")
            ps1 = pspool.tile([128, N_OUT], F32, tag="ps1")
            nc.tensor.matmul(ps0[:], s0[:, 0:128], m0, start=True, stop=False)
            nc.tensor.matmul(ps1[:], s0[:, 128:256], m0, start=True, stop=False)
            nc.tensor.matmul(ps0[:], s1[:, 0:128], m1, start=False, stop=False)
            nc.tensor.matmul(ps1[:], s1[:, 128:256], m1, start=False, stop=False)
            nc.tensor.matmul(ps0[:], ones[:], biasr, start=False, stop=False)
            nc.tensor.matmul(ps1[:], ones[:], biasr, start=False, stop=False)
            H = G // 2
            for u in range(G):
                r = rpool.tile([I_DIM, G, BC], ct, tag="r")
                # halves cut the PE's wait for the first chunk of each
                # group (gpsimd offload of these muls wedges the device --
                # NRT_EXEC_UNIT_UNRECOVERABLE -- so they stay on VectorE)
                for h in range(2):
                    nc.vector.tensor_mul(
                        r[:, h * H : (h + 1) * H, :],
                        pr[:, u : u + 1, :].broadcast_to((I_DIM, H, BC)),
                        pi[:, h * H : (h + 1) * H, :],
                    )
                for v in range(G):
                    k = u * G + v
                    wk = wqs[k // WQN][:, k % WQN, :]
                    last = k == KCH - 1
                    nc.tensor.matmul(ps0[:], r[:, v, 0:128], wk,
                                     start=False, stop=last)
                    nc.tensor.matmul(ps1[:], r[:, v, 128:256], wk,
                                     start=False, stop=last)


            # ---- BatchNorm partials: [sum_z0, sum_z1, sumsq_z0, sumsq_z1] ----
            st0 = bnpool.tile([128, 4], F32, tag="st0")
            st1 = bnpool.tile([128, 4], F32, tag="st1")
            for zi, (pst, stt) in enumerate(((ps0, st0), (ps1, st1))):
                zview = pst[:].rearrange("p (o z) -> p z o", z=D)
                for z in range(D):
                    nc.vector.tensor_reduce(stt[:, z : z + 1], zview[:, z, :],
                                            axis=mybir.AxisListType.X,
                                            op=ALU.add)
                    sqz = sqpool.tile([128, O_DIM], F32, tag="sqz")
                    nc.scalar.activation(sqz[:], zview[:, z, :], AF.Square)
                    nc.vector.tensor_reduce(stt[:, 2 + z : 3 + z], sqz[:],
                                            axis=mybir.AxisListType.X,
                                            op=ALU.add)

            # partition-sum via ones matmul (every output row = total)
            stp = pspool.tile([128, 4], F32, tag="stp")
            nc.tensor.matmul(stp[:], ones_f[:], st0[:], start=True, stop=False)
            nc.tensor.matmul(stp[:], ones_f[:], st1[:], start=False, stop=True)
            stloc = bnpool.tile([128, 4], F32, tag="stloc")
            nc.vector.tensor_copy(stloc[:], stp[:])

            if two_phase:
                # phase 1 ends here: raw (pre-norm) psum out + stats partials
                nc.sync.dma_start(st_d.ap(), stloc[0:1, :])
                for bh, pst in enumerate((ps0, ps1)):
                    ot = opool.tile([128, N_OUT], F32, tag=f"out{bh}")
                    nc.scalar.copy(ot[:], pst[:])
                    nc.sync.dma_start(
                        y_d.ap()[bh * 128 : (bh + 1) * 128, :], ot[:])
                return nc
            nc.sync.dma_start(st_loc.ap(), stloc[:])

            # ---- 2KB AllReduce of the partial stats across the 8 cores ----
            stred = bnpool.tile([128, 4], F32, tag="stred")
            with tc.tile_critical():
                cc_sem = nc.alloc_semaphore("cc_done")
                cc_dma_sem = nc.alloc_semaphore("cc_dma")
                nc.gpsimd.collective_compute(
                    "AllReduce",
                    ALU.add,
                    replica_groups=[list(range(NCORES))],
                    ins=[st_loc.ap()],
                    outs=[st_sh.ap()],
                ).then_inc(cc_sem)
                nc.gpsimd.wait_ge(cc_sem, 1)
                nc.gpsimd.dma_start(stred[:], st_sh.ap()).then_inc(cc_dma_sem, 16)
                nc.gpsimd.wait_ge(cc_dma_sem, 16)

            # ---- scale/shift per z (on all 128 partitions) ----
            mean = bnpool.tile([128, D], F32, tag="mean")
            nc.vector.tensor_scalar_mul(mean[:], stred[:, 0:2], INV_COUNT)
            msq = bnpool.tile([128, D], F32, tag="msq")
            nc.vector.tensor_scalar_mul(msq[:], stred[:, 2:4], INV_COUNT)
            var = bnpool.tile([128, D], F32, tag="var")
            nc.vector.tensor_mul(var[:], mean[:], mean[:])
            nc.vector.tensor_sub(var[:], msq[:], var[:])
            nc.vector.tensor_scalar_add(var[:], var[:], EPS)
            inv = bnpool.tile([128, D], F32, tag="inv")
            nc.vector.reciprocal(inv[:], var[:])
            nc.scalar.activation(inv[:], inv[:], AF.Sqrt)   # 1/std
            scl = bnpool.tile([128, D], F32, tag="scl")
            nc.vector.tensor_mul(scl[:], gam, inv[:])
            shf = bnpool.tile([128, D], F32, tag="shf")
            nc.vector.tensor_mul(shf[:], mean[:], scl[:])
            nc.vector.tensor_sub(shf[:], bet, shf[:])

            # ---- apply + store ----
            for bh, pst in enumerate((ps0, ps1)):
                ot = opool.tile([128, N_OUT], F32, tag=f"out{bh}")
                pv = pst[:].rearrange("p (o z) -> p z o", z=D)
                ov = ot[:].rearrange("p (o z) -> p z o", z=D)
                for z in range(D):
                    nc.scalar.activation(ov[:, z, :], pv[:, z, :], AF.Identity,
                                         bias=shf[:, z : z + 1],
                                         scale=scl[:, z : z + 1])
                nc.sync.dma_start(y_d.ap()[bh * 128 : (bh + 1) * 128, :], ot[:])

    return nc


def _prep_inputs(x, weights, silu_weight, silu_bias, gamma, beta, grid, cayley,
                 path=PATH):
    """Host-side sharding + operand layout (no math beyond folding the tiny
    cayley table into the silu weight)."""
    if path == "bf16":
        import ml_dtypes
        ctnp = ml_dtypes.bfloat16
    else:
        ctnp = np.float32

    x = np.asarray(x, np.float32)
    w2 = np.ascontiguousarray(
        np.transpose(np.asarray(weights, np.float32), (2, 3, 0, 1, 4))
    ).reshape(KCH, I_DIM, N_OUT).astype(ctnp)
    msil = np.einsum("iox,xyz->yioz", np.asarray(silu_weight, np.float32),
                     np.asarray(cayley, np.float32)).reshape(2, I_DIM, N_OUT)
    biasr = np.asarray(silu_bias, np.float32).reshape(1, I_DIM, N_OUT)
    msb = np.ascontiguousarray(
        np.concatenate([msil, biasr], axis=0)).astype(ctnp)
    onesw = np.ones((I_DIM, I_DIM), np.float32).astype(ctnp)
    g = np.asarray(grid, np.float32)
    row = np.concatenate([-g[:, 0, 0], -g[0, :, 1],
                          np.asarray(gamma, np.float32),
                          np.asarray(beta, np.float32)])
    cpack = np.ascontiguousarray(
        np.tile(row, (I_DIM, 1))[:, :, None].astype(np.float32))

    in_maps = []
    for c in range(NCORES):
        xs = x[c * BC : (c + 1) * BC]          # (BC, I, 2)
        xri = np.ascontiguousarray(
            np.stack([xs[:, :, 0].T, xs[:, :, 1].T], axis=0))
        in_maps.append({
            "xri": xri,
            "w2": w2,
            "msb": msb,
            "onesw": onesw,
            "cpack": cpack,
        })
    return in_maps


def _build_phase2():
    """Affine y = y_raw * scale[z] + shift[z] (scale/shift host-supplied)."""
    nc = bass.Bass("TRN2", target_bir_lowering=False, debug=False,
                   num_devices=NCORES)
    yr_d = nc.dram_tensor("yraw", [BC, N_OUT], F32, kind="ExternalInput")
    ss_d = nc.dram_tensor("ss", [I_DIM, 4, 1], F32, kind="ExternalInput")
    y_d = nc.dram_tensor("y", [BC, N_OUT], F32, kind="ExternalOutput")
    with _TailSplitTileContext(nc) as tc:
        with tc.tile_pool(name="p", bufs=2) as pool:
            ss = pool.tile([I_DIM, 4, 1], F32, tag="ss")
            nc.gpsimd.dma_start(ss[:], ss_d.ap())
            for bh in range(BC // 128):
                yt = pool.tile([128, N_OUT], F32, tag="y")
                nc.sync.dma_start(
                    yt[:], yr_d.ap()[bh * 128 : (bh + 1) * 128, :])
                ot = pool.tile([128, N_OUT], F32, tag="o")
                yv = yt[:].rearrange("p (o z) -> p z o", z=D)
                ov = ot[:].rearrange("p (o z) -> p z o", z=D)
                for z in range(D):
                    nc.scalar.activation(ov[:, z, :], yv[:, z, :], AF.Identity,
                                         bias=ss[:, 2 + z, :],
                                         scale=ss[:, z, :])
                nc.sync.dma_start(y_d.ap()[bh * 128 : (bh + 1) * 128, :], ot[:])
    _split_excess_waits(nc)
    return nc


def kernel(x, weights, silu_weight, silu_bias, gamma, beta, grid, cayley):
    if "nc" not in _cache:
        _cache["nc"] = _build(PATH, two_phase=TWO_PHASE)
        if TWO_PHASE:
            _cache["nc2"] = _build_phase2()
    nc = _cache["nc"]
    in_maps = _prep_inputs(x, weights, silu_weight, silu_bias, gamma, beta,
                           grid, cayley, PATH)
    res = run_bass_kernel_spmd(nc, in_maps, core_ids=list(range(NCORES)))
    if not TWO_PHASE:
        y = np.concatenate([res.results[c]["y"] for c in range(NCORES)],
                           axis=0)
        return y.reshape(B, O_DIM, D)

    # host: combine the 8 partial stat rows (32 floats) into scale/shift
    stats = np.sum([res.results[c]["stats"][0] for c in range(NCORES)], axis=0)
    mean = stats[:2] * INV_COUNT
    var = stats[2:] * INV_COUNT - mean * mean
    inv = 1.0 / np.sqrt(var + EPS)
    scale = np.asarray(gamma, np.float32) * inv
    shift = np.asarray(beta, np.float32) - mean * scale
    ss = np.tile(np.concatenate([scale, shift]).astype(np.float32),
                 (I_DIM, 1))[:, :, None]
    ss = np.ascontiguousarray(ss, dtype=np.float32)
    in2 = [{"yraw": res.results[c]["y"], "ss": ss} for c in range(NCORES)]
    res2 = run_bass_kernel_spmd(_cache["nc2"], in2,
                                core_ids=list(range(NCORES)))
    y = np.concatenate([res2.results[c]["y"] for c in range(NCORES)], axis=0)
    return y.reshape(B, O_DIM, D)

